# revision 44
# baseline (speedup 1.0000x reference)
"""LurieNet-k Trainium2 kernel.

Computes, from the raw parametrization tensors, the matrices
  C = UC @ SC @ VC^T,  B = UB @ SB @ VB^T,
  A = 0.5*UA @ SA @ UA^T + 0.5*YA  (SA = -(alpha_upp*I + GA))
entirely on device (matrix exponentials of skew matrices via
scaling-and-squaring Taylor), then runs the 511-step recurrence
  y  = C x + by
  x' = x + (0.01*A x + 0.01*B tanh(y) + 0.01*bx)
on a (128, 64) state shard per NeuronCore (batch data-parallel over the
8 cores), writing the full (b, t, n) trajectory.

Serial-chain minimization ("x-form"): the per-step critical path is
  tanh_t (ACT) -> Q @ th_t (PE) -> tanh_{t+1}
Every other matmul is re-associated one step back so it only consumes
step t-1 quantities and executes during tanh_t:
  y_{t+1} = P2 x_{t-1} + PB th_{t-1} + Q th_t + r2
  x_{t+1} = x_t + (A2 x_{t-1} + AB th_{t-1} + B' th_t) + bx2
with M = I + 0.01A, B' = 0.01B, P = C M, Q = C B',
  P2 = P M, PB = P B', r2 = r + P bxp,
  A2 = 0.01 A M, AB = 0.01 A B', bx2 = M bxp,  bxp = 0.01 bx.

Precision: split-carry — the fp32 state is carried outside the matmuls
(DVE STT from the psx accumulator), with a rounded bf16 copy feeding
the PE (gains through the matmuls are <= ~0.3 so bf16 operands cost
~2e-3 final rel err; bf16 weights also get FWL weight loads, keeping
the PE off the LDWEIGHTS-throughput wall). The PSUM->SBUF output
drains run on DVE (not ACT) so a drain can never queue-block the
chain-critical tanh.
"""

import sys

for _p in ("/opt/trn_rl_repo",):
    if _p not in sys.path:
        sys.path.insert(0, _p)

import numpy as np

import concourse.bass as bass
import concourse.mybir as mybir
import concourse.tile as tile
from concourse import bacc
from concourse import bass_isa
from concourse.bass import ds
from concourse.bass_utils import run_bass_kernel_spmd

F32 = mybir.dt.float32
F32R = mybir.dt.float32r
BF16 = mybir.dt.bfloat16
FP16 = mybir.dt.float16
ALU = mybir.AluOpType
ACTF = mybir.ActivationFunctionType
AXIS = mybir.AxisListType

N = 128          # state dim
TMAX = 512       # time steps (including t=0)
BS = 512         # global batch
NCORES = 8
BSH = BS // NCORES   # 64 batch columns per core
STEP = 0.01
KTOP = 4

EXPM_SCAL = 2    # expm scaling: X = S / 2**EXPM_SCAL, then 2 squarings
EXPM_TERMS = 5   # Taylor terms in the Horner evaluation

PARAM_NAMES = [
    "ZC_U", "ZC_V", "ZB_U", "ZB_V", "ZA_U", "ZC_S", "ZB_S", "ZA_G", "ZA_Y",
]


def build_program(tmax=TMAX, tc_chunk=32, mdt=BF16):
    """Build the single-NeuronCore Bass program (run SPMD on all 8 cores).

    mdt: dtype of the recurrence matmul operands (weights, rounded state
    copy, tanh output). The fp32 state carry is exact regardless.
    """
    assert tmax % tc_chunk == 0 and tc_chunk % 2 == 0
    half = tc_chunk // 2
    nchunks = tmax // tc_chunk

    nc = bacc.Bacc(
        "TRN2",
        target_bir_lowering=False,
        debug=False,
        enable_asserts=False,
        num_devices=NCORES,
    )

    # two packed inputs: one DMA issue each instead of 13 (DMA-issue
    # serialization on the sync queue otherwise delays the expm start)
    # zall: the 9 parameter matrices side by side; misc: ident | masku |
    # x0 | bx | by
    zall = nc.dram_tensor("zall", [N, 9 * N], F32, kind="ExternalInput")
    misc = nc.dram_tensor("misc", [N, 2 * N + BSH + 2], F32, kind="ExternalInput")
    out = nc.dram_tensor("out", [BSH, tmax, N], F32, kind="ExternalOutput")

    with tile.TileContext(nc) as tc:
        with tc.tile_pool(name="const", bufs=1) as constp:
            # identity/mask constants come in as DMA inputs: building them
            # with gpsimd iota ops costs ~5us of drain-fenced preamble
            miscs = constp.tile([N, 2 * N + BSH + 2], F32, tag="miscs")
            nc.sync.dma_start(out=miscs[:], in_=misc[:])
            ident = miscs[:, 0:N]
            masku = miscs[:, ds(N, N)]
            x0_c = miscs[:, ds(2 * N, BSH)]
            bx_c = miscs[:, ds(2 * N + BSH, 1)]
            by_c = miscs[:, ds(2 * N + BSH + 1, 1)]
            ident_h = constp.tile([N, N], FP16, tag="ident_h")
            nc.vector.tensor_copy(ident_h[:], ident)
            ident_b = constp.tile([N, N], BF16, tag="ident_b")
            nc.vector.tensor_copy(ident_b[:], ident)
            # scaled identities for the integer-scaled Horner recurrence
            # w_k = c_k I + X w_{k-1} (c_k = j_k c_{k-1}): the c_k I term is
            # added by an identity matmul into the psum group, so the
            # per-term DVE scalar_tensor_tensor collapses to a plain copy
            # that can run on either DVE or ACT.
            horner_cs = []
            ck = 1.0
            for j in range(EXPM_TERMS, 0, -1):
                ck *= j
                horner_cs.append(ck)
            ident_cs = {}
            for cval in sorted(set(horner_cs)):
                tl = constp.tile([N, N], FP16, tag=f"identc_{int(cval)}")
                nc.vector.tensor_scalar_mul(tl[:], ident_h[:], cval)
                ident_cs[cval] = tl

            bxp_c = constp.tile([N, 1], F32, tag="bxp")
            nc.vector.tensor_scalar_mul(bxp_c[:], bx_c, STEP)

            # ------- setup phase: expm's + weight assembly -------
            # Weights for the x-form recurrence (all stored transposed, as
            # matmul lhsT):
            PTf = constp.tile([N, N], F32, tag="PTf")      # P^T fp32
            P2Tm = constp.tile([N, N], mdt, tag="P2Tm")    # (P M)^T
            PBTm = constp.tile([N, N], mdt, tag="PBTm")    # (P B')^T
            QTm = constp.tile([N, N], mdt, tag="QTm")      # Q^T
            A2Tm = constp.tile([N, N], mdt, tag="A2Tm")    # (0.01A M)^T
            ABTm = constp.tile([N, N], mdt, tag="ABTm")    # (0.01A B')^T
            BpTm = constp.tile([N, N], mdt, tag="BpTm")    # (0.01 B)^T
            A01Tm = constp.tile([N, N], mdt, tag="A01Tm")  # (0.01 A)^T
            CTf32 = constp.tile([N, N], F32, tag="CTf32")  # C^T fp32
            r_c = constp.tile([N, 1], F32, tag="rc")       # 0.01 C bx + by
            r2_c = constp.tile([N, 1], F32, tag="r2c")     # r + P bxp
            bx2_c = constp.tile([N, 1], F32, tag="bx2c")   # (I + 0.01A) bxp

            with (
                tc.tile_pool(name="zbuf", bufs=1) as zp,
                tc.tile_pool(name="work", bufs=2) as wp,
                tc.tile_pool(name="eres", bufs=1) as ep,
                tc.tile_pool(name="small", bufs=1) as sp,
                tc.tile_pool(name="pss", bufs=4, space="PSUM") as psp,
            ):
                zalls = zp.tile([N, 9 * N], F32, tag="zalls")
                nc.scalar.dma_start(
                    out=zalls[:, 0:5 * N], in_=zall[:, 0:5 * N]
                )
                nc.sync.dma_start(
                    out=zalls[:, ds(5 * N, 4 * N)], in_=zall[:, ds(5 * N, 4 * N)]
                )
                zt = {
                    name: zalls[:, ds(k * N, N)]
                    for k, name in enumerate(PARAM_NAMES)
                }

                def expm_batch(specs, side_work=()):
                    """Interleaved expm(skew(Z))^T for all matrices at once.

                    Maintains the (T, T^T) pair through Horner + squaring so
                    no PE transposes are needed: with negX = X^T = -X,
                      X @ T     = matmul(lhsT=negX, rhs=T)
                      T^T @ X^T = matmul(lhsT=T,    rhs=negX)
                    The five chains are advanced stage-by-stage so PE/DVE/ACT
                    work from different chains overlaps (a single sequential
                    chain is latency-bound on the MM->STT->MM dependency).
                    """
                    scal = 1.0 / (2.0 ** EXPM_SCAL)
                    negx = {}
                    t_cur = {}
                    tt_cur = {}
                    for z_tile, tag in specs:
                        us = wp.tile([N, N], FP16, tag="us_r", name=f"us_{tag}")
                        nc.vector.scalar_tensor_tensor(
                            us[:], z_tile, scal, masku,
                            op0=ALU.mult, op1=ALU.mult,
                        )
                        pst = psp.tile([N, N], FP16, tag="ps", bufs=8,
                                       name=f"pst_{tag}")
                        nc.tensor.transpose(pst[:], us[:], ident_h[:])
                        nx = wp.tile([N, N], FP16, tag=f"negx_{tag}", bufs=1,
                                     name=f"negx_{tag}")
                        nc.vector.scalar_tensor_tensor(
                            nx[:], pst[:], 1.0, us[:],
                            op0=ALU.mult, op1=ALU.subtract,
                        )
                        negx[tag] = nx
                        t_cur[tag] = ident_h
                        tt_cur[tag] = ident_h
                    # Integer-scaled Horner: w_k = c_k I + X w_{k-1}
                    # (w_0 = I, final w = c_last * T). The c_k I term comes
                    # from an identity matmul accumulated into the psum
                    # group, so each term needs only a plain psum->sbuf
                    # copy, alternated across DVE and ACT (DVE is the
                    # setup-phase bottleneck otherwise). The 1/c_last
                    # normalization folds into the first squaring's copy
                    # scale.
                    nsplit = 0
                    for k, j in enumerate(range(EXPM_TERMS, 0, -1)):
                        cval = horner_cs[k]
                        for _, tag in specs:
                            psa = psp.tile([N, N], F32, tag="ps", bufs=8)
                            nc.tensor.matmul(
                                psa[:], ident_h[:], ident_cs[cval][:],
                                start=True, stop=False,
                            )
                            nc.tensor.matmul(
                                psa[:], negx[tag][:], t_cur[tag][:],
                                start=False, stop=True,
                            )
                            t_new = wp.tile([N, N], FP16, tag=f"T_{tag}",
                                            bufs=2, name=f"T_{tag}")
                            if nsplit % 2 == 0:
                                nc.vector.tensor_copy(t_new[:], psa[:])
                            else:
                                nc.scalar.copy(t_new[:], psa[:])
                            nsplit += 1
                            t_cur[tag] = t_new
                        if side_work:
                            side_work.pop(0)()
                    for _, tag in specs:
                        pst = psp.tile([N, N], FP16, tag="ps", bufs=8,
                                       name=f"ptt_{tag}")
                        nc.tensor.transpose(pst[:], t_cur[tag][:], ident_h[:])
                        tt_new = wp.tile([N, N], FP16, tag=f"TT_{tag}",
                                         bufs=2, name=f"TT_{tag}")
                        nc.scalar.copy(tt_new[:], pst[:])
                        tt_cur[tag] = tt_new
                    wnorm = 1.0 / (horner_cs[-1] ** 2)
                    for sq in range(EXPM_SCAL):
                        for _, tag in specs:
                            psa = psp.tile([N, N], F32, tag="ps", bufs=8)
                            psb = psp.tile([N, N], F32, tag="ps", bufs=8)
                            nc.tensor.matmul(
                                psa[:], tt_cur[tag][:], t_cur[tag][:],
                                start=True, stop=True,
                            )
                            nc.tensor.matmul(
                                psb[:], t_cur[tag][:], tt_cur[tag][:],
                                start=True, stop=True,
                            )
                            t_new = wp.tile([N, N], FP16, tag=f"T_{tag}",
                                            bufs=2, name=f"T_{tag}")
                            tt_new = wp.tile([N, N], FP16, tag=f"TT_{tag}",
                                             bufs=2, name=f"TT_{tag}")
                            if sq == 0:
                                # both operands carried the c_last factor
                                nc.vector.tensor_scalar_mul(
                                    t_new[:], psa[:], wnorm
                                )
                                nc.scalar.activation(
                                    tt_new[:], psb[:], ACTF.Copy, scale=wnorm
                                )
                            else:
                                nc.vector.tensor_copy(t_new[:], psa[:])
                                nc.scalar.copy(tt_new[:], psb[:])
                            t_cur[tag], tt_cur[tag] = t_new, tt_new
                        if side_work:
                            side_work.pop(0)()
                    while side_work:
                        side_work.pop(0)()
                    return tt_cur

                # top-4 alpha pipeline (absdiag -> 4 max-extractions ->
                # sqrt): a serial DVE+GpSimd chain. Issued as side_work
                # hooks between expm Horner/squaring stages so it overlaps
                # the expm instead of stalling the A-matrix assembly after.
                dc_col = sp.tile([N, 1], F32, tag="dc")
                db_col = sp.tile([N, 1], F32, tag="db")
                ga_col = sp.tile([N, 1], F32, tag="ga")
                bwork = sp.tile([N, 1], F32, tag="bwork")
                cwork = sp.tile([N, 1], F32, tag="cwork")
                acc = sp.tile([N, 1], F32, tag="acc")
                bmax = sp.tile([N, 1], F32, tag="bmax")
                cmax = sp.tile([N, 1], F32, tag="cmax")
                prod = sp.tile([N, 1], F32, tag="prod")
                gmask = sp.tile([N, 1], F32, tag="gmask")
                tdrop = sp.tile([N, 1], F32, tag="tdrop")
                alpha = sp.tile([N, 1], F32, tag="alpha")
                sa05 = sp.tile([N, 1], F32, tag="sa05")
                sb01 = sp.tile([N, 1], F32, tag="sb01")

                def topk_absdiag():
                    for z_ap, col in (
                        (zt["ZC_S"], dc_col), (zt["ZB_S"], db_col),
                        (zt["ZA_G"], ga_col),
                    ):
                        tmp = wp.tile([N, N], F32, tag="us")
                        nc.vector.tensor_mul(tmp[:], z_ap, ident)
                        nc.vector.tensor_reduce(
                            col[:], tmp[:], AXIS.X, ALU.add,
                            apply_absolute_value=True,
                        )
                    nc.vector.tensor_copy(bwork[:], db_col[:])
                    nc.vector.tensor_copy(cwork[:], dc_col[:])
                    nc.vector.memset(acc[:], 0.0)

                def topk_iter(i):
                    def go():
                        nc.gpsimd.partition_all_reduce(
                            bmax[:], bwork[:], N, bass_isa.ReduceOp.max
                        )
                        nc.gpsimd.partition_all_reduce(
                            cmax[:], cwork[:], N, bass_isa.ReduceOp.max
                        )
                        nc.gpsimd.tensor_mul(prod[:], bmax[:], cmax[:])
                        nc.gpsimd.tensor_mul(prod[:], prod[:], prod[:])
                        nc.gpsimd.tensor_add(acc[:], acc[:], prod[:])
                        if i < KTOP - 1:
                            # zero out the extracted max (values all > 0)
                            nc.gpsimd.tensor_single_scalar(
                                gmask[:], bwork[:], bmax[:], ALU.is_ge
                            )
                            nc.gpsimd.tensor_mul(tdrop[:], bwork[:], gmask[:])
                            nc.gpsimd.tensor_sub(bwork[:], bwork[:], tdrop[:])
                            nc.gpsimd.tensor_single_scalar(
                                gmask[:], cwork[:], cmax[:], ALU.is_ge
                            )
                            nc.gpsimd.tensor_mul(tdrop[:], cwork[:], gmask[:])
                            nc.gpsimd.tensor_sub(cwork[:], cwork[:], tdrop[:])
                    return go

                def topk_final():
                    # alpha = sqrt(sum_i (b_i c_i)^2) over the top-4 pairs
                    nc.scalar.activation(alpha[:], acc[:], ACTF.Sqrt)
                    # sa05 = -0.5*(alpha + gA) (per-partition row scale)
                    nc.vector.tensor_scalar(
                        sa05[:], ga_col[:], alpha[:], -0.5,
                        op0=ALU.add, op1=ALU.mult,
                    )
                    nc.vector.tensor_scalar_mul(sb01[:], db_col[:], STEP)

                eres = expm_batch(
                    [
                        (zt["ZC_U"], "UCT"), (zt["ZC_V"], "VCT"),
                        (zt["ZB_U"], "UBT"), (zt["ZB_V"], "VBT"),
                        (zt["ZA_U"], "UAT"),
                    ],
                    side_work=[topk_absdiag]
                    + [topk_iter(i) for i in range(KTOP)]
                    + [topk_final],
                )
                uct, vct = eres["UCT"], eres["VCT"]
                ubt, vbt = eres["UBT"], eres["VBT"]
                uat = eres["UAT"]

                # C^T = VC @ (SC @ UC^T)
                p1 = wp.tile([N, N], FP16, tag="us_r", name="p1")
                nc.vector.tensor_scalar_mul(p1[:], uct[:], dc_col[:])
                psa = psp.tile([N, N], F32, tag="ps", bufs=8)
                nc.tensor.matmul(psa[:], vct[:], p1[:], start=True, stop=True)
                nc.vector.tensor_copy(CTf32[:], psa[:])
                ct_h = ep.tile([N, N], FP16, tag="ct_h")
                nc.scalar.copy(ct_h[:], psa[:])

                # (0.01 B)^T = VB @ (0.01 SB @ UB^T)
                p2 = wp.tile([N, N], FP16, tag="us_r", name="p2")
                nc.vector.tensor_scalar_mul(p2[:], ubt[:], sb01[:])
                psb = psp.tile([N, N], F32, tag="ps", bufs=8)
                nc.tensor.matmul(psb[:], vbt[:], p2[:], start=True, stop=True)
                nc.vector.tensor_copy(BpTm[:], psb[:])
                # untransposed 0.01 B = UB @ (0.01 SB @ VB^T)
                p2b = wp.tile([N, N], FP16, tag="us_r", name="p2b")
                nc.vector.tensor_scalar_mul(p2b[:], vbt[:], sb01[:])
                psb2 = psp.tile([N, N], F32, tag="ps", bufs=8)
                nc.tensor.matmul(psb2[:], ubt[:], p2b[:], start=True, stop=True)
                bp_h = ep.tile([N, N], FP16, tag="Bpun")
                nc.vector.tensor_copy(bp_h[:], psb2[:])

                # M = UA @ (sa05 * UA^T) = 0.5*UA SA UA^T (symmetric)
                p3 = wp.tile([N, N], FP16, tag="us_r", name="p3")
                nc.vector.tensor_scalar_mul(p3[:], uat[:], sa05[:])
                psm = psp.tile([N, N], F32, tag="ps", bufs=8)
                nc.tensor.matmul(psm[:], uat[:], p3[:], start=True, stop=True)
                # YA = Uy - Uy^T; q2 = -0.005*YA
                uy = wp.tile([N, N], FP16, tag="us")
                nc.vector.tensor_mul(uy[:], zt["ZA_Y"], masku)
                pst2 = psp.tile([N, N], FP16, tag="ps", bufs=8)
                nc.tensor.transpose(pst2[:], uy[:], ident_h[:])
                nc.vector.tensor_scalar_mul(uy[:], uy[:], 0.5 * STEP)
                q2 = wp.tile([N, N], F32, tag="T")
                nc.vector.scalar_tensor_tensor(
                    q2[:], pst2[:], 0.5 * STEP, uy[:], op0=ALU.mult, op1=ALU.subtract
                )
                # (0.01 A)^T = 0.01*M + q2 ; untransposed 0.01 A = 0.01*M - q2
                a01tf = ep.tile([N, N], F32, tag="A01Tf")
                nc.vector.scalar_tensor_tensor(
                    a01tf[:], psm[:], STEP, q2[:], op0=ALU.mult, op1=ALU.add
                )
                nc.vector.tensor_copy(A01Tm[:], a01tf[:])
                a01t_h = ep.tile([N, N], FP16, tag="A01Th")
                nc.vector.tensor_copy(a01t_h[:], a01tf[:])
                a01h_un = ep.tile([N, N], FP16, tag="A01un")
                nc.vector.scalar_tensor_tensor(
                    a01h_un[:], psm[:], STEP, q2[:], op0=ALU.mult, op1=ALU.subtract
                )
                bxp_h = sp.tile([N, 1], FP16, tag="bxp_h")
                nc.vector.tensor_copy(bxp_h[:], bxp_c[:])

                # All weight-product matmuls run on fp16 operands (one FWL
                # weight load instead of fp32's two serialized 333ns LOW/
                # HIGH loads); the fp32 additive combines keep the dominant
                # terms exact, and everything here lands in bf16 anyway.
                # P^T = C^T + (0.01 A)^T C^T ;  Q^T = (0.01 B)^T C^T
                psw = psp.tile([N, N], F32, tag="ps", bufs=8)
                nc.tensor.matmul(psw[:], a01h_un[:], ct_h[:], start=True, stop=True)
                nc.vector.scalar_tensor_tensor(
                    PTf[:], psw[:], 1.0, CTf32[:], op0=ALU.mult, op1=ALU.add
                )
                pt_h = ep.tile([N, N], FP16, tag="pt_h")
                nc.vector.tensor_copy(pt_h[:], PTf[:])
                psq = psp.tile([N, N], F32, tag="ps", bufs=8)
                nc.tensor.matmul(psq[:], bp_h[:], ct_h[:], start=True, stop=True)
                nc.vector.tensor_copy(QTm[:], psq[:])

                # P2^T = P^T + (0.01A)^T P^T ; PB^T = (0.01B)^T P^T
                ps2 = psp.tile([N, N], F32, tag="ps", bufs=8)
                nc.tensor.matmul(ps2[:], a01h_un[:], pt_h[:], start=True, stop=True)
                nc.vector.scalar_tensor_tensor(
                    P2Tm[:], ps2[:], 1.0, PTf[:], op0=ALU.mult, op1=ALU.add
                )
                pspb = psp.tile([N, N], F32, tag="ps", bufs=8)
                nc.tensor.matmul(pspb[:], bp_h[:], pt_h[:], start=True, stop=True)
                nc.vector.tensor_copy(PBTm[:], pspb[:])

                # A2^T = (0.01A)^T + ((0.01A)^2)^T ; AB^T = (0.01B)^T (0.01A)^T
                psa2 = psp.tile([N, N], F32, tag="ps", bufs=8)
                nc.tensor.matmul(psa2[:], a01h_un[:], a01t_h[:], start=True, stop=True)
                nc.vector.scalar_tensor_tensor(
                    A2Tm[:], psa2[:], 1.0, a01tf[:], op0=ALU.mult, op1=ALU.add
                )
                psab = psp.tile([N, N], F32, tag="ps", bufs=8)
                nc.tensor.matmul(psab[:], bp_h[:], a01t_h[:], start=True, stop=True)
                nc.vector.tensor_copy(ABTm[:], psab[:])

                # r = 0.01 C bx + by ; r2 = r + P bxp ; bx2 = bxp + 0.01A bxp
                psr = psp.tile([N, 1], F32, tag="ps", bufs=8, name="psr")
                nc.tensor.matmul(psr[:], ct_h[:], bxp_h[:], start=True, stop=True)
                nc.vector.scalar_tensor_tensor(
                    r_c[:], psr[:], 1.0, by_c, op0=ALU.mult, op1=ALU.add
                )
                psr2 = psp.tile([N, 1], F32, tag="ps", bufs=8, name="psr2")
                nc.tensor.matmul(psr2[:], pt_h[:], bxp_h[:], start=True, stop=True)
                nc.vector.scalar_tensor_tensor(
                    r2_c[:], psr2[:], 1.0, r_c[:], op0=ALU.mult, op1=ALU.add
                )
                psbx = psp.tile([N, 1], F32, tag="ps", bufs=8, name="psbx")
                nc.tensor.matmul(psbx[:], a01t_h[:], bxp_h[:], start=True, stop=True)
                nc.vector.scalar_tensor_tensor(
                    bx2_c[:], psbx[:], 1.0, bxp_c[:], op0=ALU.mult, op1=ALU.add
                )

            # ------- recurrence (split-carry, x-form) -------
            with (
                tc.tile_pool(name="xrb", bufs=2) as xrbp,
                tc.tile_pool(name="xc", bufs=3) as xcp,
                tc.tile_pool(name="stage", bufs=2) as stagep,
                tc.tile_pool(name="th", bufs=3) as thp,
                tc.tile_pool(name="psy", bufs=3, space="PSUM") as psyp,
                tc.tile_pool(name="psx", bufs=3, space="PSUM") as psxp,
                tc.tile_pool(name="pstr", bufs=2, space="PSUM") as pstrp,
            ):
                # The rounded bf16 state copies live in a chunk buffer with
                # consecutive steps adjacent, so each pair-transpose covers
                # steps (s-1, s): transposes spread one-per-two-cycles over
                # the whole chunk, run in bf16 (FWL ident load, 1 cyc/row)
                # and so never crowd the chain-critical Q matmul. The fp32
                # carry never needs transposing and stays in a small pool.
                # th_0 = tanh(C x_0 + by), fp32 matmul (one-time)
                psy0 = psyp.tile([N, BSH], F32, tag="psy", name="psy0")
                nc.tensor.matmul(psy0[:], CTf32[:], x0_c, start=True, stop=True)
                th_cur = thp.tile([N, BSH], mdt, tag="th", name="th_init")
                nc.scalar.activation(
                    th_cur[:], psy0[:], ACTF.Tanh, bias=by_c, scale=1.0
                )
                xc_prev = xcp.tile([N, BSH], F32, tag="xc", name="xc0")
                nc.vector.tensor_copy(xc_prev[:], x0_c)
                xrb0 = xrbp.tile([N, tc_chunk * BSH], mdt, tag="xrb")
                nc.vector.tensor_copy(xrb0[:, 0:BSH], x0_c)
                xr_old = xrb0[:, 0:BSH]  # xr_{t-1} at iteration t=1
                # psy_1 = P x_0 + Q th_0 (P-form, one-time; P @ x_0 in fp32)
                psy_t = psyp.tile([N, BSH], F32, tag="psy", name="psy1")
                nc.tensor.matmul(psy_t[:], PTf[:], x0_c, start=True, stop=False)
                nc.tensor.matmul(psy_t[:], QTm[:], th_cur[:], start=False, stop=True)
                # psx_1 = 0.01A x_0 + 0.01B th_0 (P-form, one-time)
                psx_t = psxp.tile([N, BSH], F32, tag="psx", name="psx1")
                nc.tensor.matmul(psx_t[:], A01Tm[:], xr_old, start=True, stop=False)
                nc.tensor.matmul(psx_t[:], BpTm[:], th_cur[:], start=False, stop=True)

                th_old = th_cur  # th_0 (th_{t-1} at iteration t=1)

                xrb_prev = None
                for c in range(nchunks):
                    xrb = xrb0 if c == 0 else xrbp.tile(
                        [N, tc_chunk * BSH], mdt, tag="xrb"
                    )
                    st = stagep.tile([128, half * N], F32, tag="st")
                    for s in range(tc_chunk):
                        t = c * tc_chunk + s
                        if t > 0:
                            xslot = xrb[:, ds(s * BSH, BSH)]
                            # chain-critical: th_t = tanh(psy_t + r)
                            th_new = thp.tile([N, BSH], mdt, tag="th")
                            nc.scalar.activation(
                                th_new[:], psy_t[:], ACTF.Tanh,
                                bias=(r_c[:] if t == 1 else r2_c[:]), scale=1.0,
                            )
                            # exact fp32 state carry
                            xc_new = xcp.tile([N, BSH], F32, tag="xc")
                            nc.vector.scalar_tensor_tensor(
                                xc_new[:], psx_t[:],
                                bxp_c[:] if t == 1 else bx2_c[:],
                                xc_prev[:], op0=ALU.add, op1=ALU.add,
                            )
                            xc_prev = xc_new
                            # rounded copy for matmuls + output transposes
                            # (cheap SBUF->SBUF DVE op, off the chain;
                            # NOT on Pool -- a Pool cast costs ~450ns and
                            # paces the whole loop to ~930ns/step)
                            nc.vector.tensor_copy(xslot, xc_new[:])
                            xr_new = xslot
                            if t < tmax - 1:
                                # psy_{t+1} / psx_{t+1}: the four t-1
                                # partials execute during tanh_t; only
                                # Q@th_t / B'@th_t wait on the tanh.
                                psy_n = psyp.tile([N, BSH], F32, tag="psy")
                                psx_n = psxp.tile([N, BSH], F32, tag="psx")
                                nc.tensor.matmul(
                                    psy_n[:], P2Tm[:], xr_old,
                                    start=True, stop=False,
                                )
                                nc.tensor.matmul(
                                    psx_n[:], A2Tm[:], xr_old,
                                    start=True, stop=False,
                                )
                                nc.tensor.matmul(
                                    psy_n[:], PBTm[:], th_old[:],
                                    start=False, stop=False,
                                )
                                nc.tensor.matmul(
                                    psx_n[:], ABTm[:], th_old[:],
                                    start=False, stop=False,
                                )
                                nc.tensor.matmul(
                                    psy_n[:], QTm[:], th_new[:],
                                    start=False, stop=True,
                                )
                                nc.tensor.matmul(
                                    psx_n[:], BpTm[:], th_new[:],
                                    start=False, stop=True,
                                )
                                psy_t, psx_t = psy_n, psx_n
                            xr_old = xr_new
                            th_old = th_new
                        if s % 2 == 1:
                            j = s // 2
                            # transpose steps (s-1, s): one contiguous
                            # 128-col block. Two pair-transposes share one
                            # psum tile; a single DVE copy drains both
                            # (keeps the drain off ACT so it can't
                            # queue-block the chain-critical tanh).
                            if j % 2 == 0:
                                pstr = pstrp.tile([128, 2 * N], mdt, tag="pstr")
                            nc.tensor.transpose(
                                pstr[:, ds((j % 2) * N, N)],
                                xrb[:, ds((s - 1) * BSH, 2 * BSH)],
                                ident_b[:],
                            )
                            if j % 2 == 1 or s == tc_chunk - 1:
                                lo = (j - (j % 2)) * N
                                width = (j % 2 + 1) * N
                                nc.vector.tensor_copy(
                                    st[:, ds(lo, width)], pstr[:, 0:width]
                                )
                    # st col-block j holds steps (2j, 2j+1): partitions
                    # 0-63 = even step's batch, 64-127 = odd step's, so a
                    # partition-half DMA writes stride-2 time indices.
                    qn = max(half // 4, 1)
                    ndma = 0
                    for h in range(2):
                        for q0 in range(0, half, qn):
                            t0 = c * tc_chunk + h + 2 * q0
                            eng = nc.sync
                            if c == nchunks - 1 and ndma % 2 == 1:
                                eng = nc.scalar
                            eng.dma_start(
                                out=out[:, t0:t0 + 2 * qn - 1:2, :],
                                in_=st[h * 64:(h + 1) * 64, ds(q0 * N, qn * N)],
                            )
                            ndma += 1
                    xrb_prev = xrb

    nc.compile()
    return nc


_CACHED = {}


def _get_program(tmax=TMAX, tc_chunk=32, mdt=BF16):
    key = (tmax, tc_chunk, str(mdt))
    if key not in _CACHED:
        _CACHED[key] = build_program(tmax, tc_chunk, mdt)
    return _CACHED[key]


def make_in_maps(inputs, tmax=TMAX):
    X0 = np.ascontiguousarray(np.asarray(inputs["X0"], dtype=np.float32))
    zall = np.concatenate(
        [np.asarray(inputs[name], dtype=np.float32) for name in PARAM_NAMES],
        axis=1,
    )
    base = {"zall": np.ascontiguousarray(zall)}
    in_maps = []
    for c in range(NCORES):
        m = dict(base)
        misc = np.concatenate(
            [
                np.eye(N, dtype=np.float32),
                np.triu(np.ones((N, N), dtype=np.float32), 1),
                X0[c * BSH:(c + 1) * BSH].T,
                np.asarray(inputs["bx"], dtype=np.float32).reshape(N, 1),
                np.asarray(inputs["by"], dtype=np.float32).reshape(N, 1),
            ],
            axis=1,
        )
        m["misc"] = np.ascontiguousarray(misc)
        in_maps.append(m)
    return in_maps


def run_spmd(inputs, tmax=TMAX, tc_chunk=32, trace=False, tmpdir=None, mdt=BF16):
    nc = _get_program(tmax, tc_chunk, mdt)
    in_maps = make_in_maps(inputs, tmax)
    res = run_bass_kernel_spmd(
        nc, in_maps, list(range(NCORES)), trace=trace, tmpdir=tmpdir
    )
    outs = [res.results[c]["out"] for c in range(NCORES)]
    full = np.concatenate(outs, axis=0)
    return full, res


def kernel(**inputs):
    full, _ = run_spmd(inputs)
    return full


# revision 45
# speedup vs baseline: 1.2082x; 1.2082x over previous
"""LurieNet-k Trainium2 kernel.

Computes, from the raw parametrization tensors, the matrices
  C = UC @ SC @ VC^T,  B = UB @ SB @ VB^T,
  A = 0.5*UA @ SA @ UA^T + 0.5*YA  (SA = -(alpha_upp*I + GA))
entirely on device (matrix exponentials of skew matrices via
scaling-and-squaring Taylor), then runs the 511-step recurrence
  y  = C x + by
  x' = x + (0.01*A x + 0.01*B tanh(y) + 0.01*bx)
on a (128, 64) state shard per NeuronCore (batch data-parallel over the
8 cores), writing the full (b, t, n) trajectory.

Serial-chain minimization ("x-form"): the per-step critical path is
  tanh_t (ACT) -> Q @ th_t (PE) -> tanh_{t+1}
Every other matmul is re-associated one step back so it only consumes
step t-1 quantities and executes during tanh_t:
  y_{t+1} = P2 x_{t-1} + PB th_{t-1} + Q th_t + r2
  x_{t+1} = x_t + (A2 x_{t-1} + AB th_{t-1} + B' th_t) + bx2
with M = I + 0.01A, B' = 0.01B, P = C M, Q = C B',
  P2 = P M, PB = P B', r2 = r + P bxp,
  A2 = 0.01 A M, AB = 0.01 A B', bx2 = M bxp,  bxp = 0.01 bx.

Precision: split-carry — the fp32 state is carried outside the matmuls
(DVE STT from the psx accumulator), with a rounded bf16 copy feeding
the PE (gains through the matmuls are <= ~0.3 so bf16 operands cost
~2e-3 final rel err; bf16 weights also get FWL weight loads, keeping
the PE off the LDWEIGHTS-throughput wall). The PSUM->SBUF output
drains run on DVE (not ACT) so a drain can never queue-block the
chain-critical tanh.
"""

import sys

for _p in ("/opt/trn_rl_repo",):
    if _p not in sys.path:
        sys.path.insert(0, _p)

import numpy as np

import concourse.bass as bass
import concourse.mybir as mybir
import concourse.tile as tile
from concourse import bacc
from concourse import bass_isa
from concourse.bass import ds
from concourse.bass_utils import run_bass_kernel_spmd

F32 = mybir.dt.float32
F32R = mybir.dt.float32r
BF16 = mybir.dt.bfloat16
FP16 = mybir.dt.float16
ALU = mybir.AluOpType
ACTF = mybir.ActivationFunctionType
AXIS = mybir.AxisListType

N = 128          # state dim
TMAX = 512       # time steps (including t=0)
BS = 512         # global batch
NCORES = 8
BSH = BS // NCORES   # 64 batch columns per core
STEP = 0.01
KTOP = 4

EXPM_SCAL = 2    # expm scaling: X = S / 2**EXPM_SCAL, then 2 squarings
EXPM_TERMS = 5   # Taylor terms in the Horner evaluation

PARAM_NAMES = [
    "ZC_U", "ZC_V", "ZB_U", "ZB_V", "ZA_U", "ZC_S", "ZB_S", "ZA_G", "ZA_Y",
]


def build_program(tmax=TMAX, tc_chunk=32, mdt=BF16):
    """Build the single-NeuronCore Bass program (run SPMD on all 8 cores).

    mdt: dtype of the recurrence matmul operands (weights, rounded state
    copy, tanh output). The fp32 state carry is exact regardless.
    """
    assert tmax % tc_chunk == 0 and tc_chunk % 2 == 0
    half = tc_chunk // 2
    nchunks = tmax // tc_chunk

    nc = bacc.Bacc(
        "TRN2",
        target_bir_lowering=False,
        debug=False,
        enable_asserts=False,
        num_devices=NCORES,
    )

    # two packed inputs: one DMA issue each instead of 13 (DMA-issue
    # serialization on the sync queue otherwise delays the expm start)
    # zall: the 9 parameter matrices side by side; misc: ident | masku |
    # x0 | bx | by
    zall = nc.dram_tensor("zall", [N, 9 * N], F32, kind="ExternalInput")
    misc = nc.dram_tensor("misc", [N, 2 * N + BSH + 2], F32, kind="ExternalInput")
    out = nc.dram_tensor("out", [BSH, tmax, N], F32, kind="ExternalOutput")

    with tile.TileContext(nc) as tc:
        with tc.tile_pool(name="const", bufs=1) as constp:
            # identity/mask constants come in as DMA inputs: building them
            # with gpsimd iota ops costs ~5us of drain-fenced preamble
            miscs = constp.tile([N, 2 * N + BSH + 2], F32, tag="miscs")
            nc.sync.dma_start(out=miscs[:], in_=misc[:])
            ident = miscs[:, 0:N]
            masku = miscs[:, ds(N, N)]
            x0_c = miscs[:, ds(2 * N, BSH)]
            bx_c = miscs[:, ds(2 * N + BSH, 1)]
            by_c = miscs[:, ds(2 * N + BSH + 1, 1)]
            ident_h = constp.tile([N, N], FP16, tag="ident_h")
            nc.vector.tensor_copy(ident_h[:], ident)
            ident_b = constp.tile([N, N], BF16, tag="ident_b")
            nc.vector.tensor_copy(ident_b[:], ident)
            # scaled identities for the integer-scaled Horner recurrence
            # w_k = c_k I + X w_{k-1} (c_k = j_k c_{k-1}): the c_k I term is
            # added by an identity matmul into the psum group, so the
            # per-term DVE scalar_tensor_tensor collapses to a plain copy
            # that can run on either DVE or ACT.
            horner_cs = []
            ck = 1.0
            for j in range(EXPM_TERMS, 0, -1):
                ck *= j
                horner_cs.append(ck)
            ident_cs = {}
            for cval in sorted(set(horner_cs)):
                tl = constp.tile([N, N], FP16, tag=f"identc_{int(cval)}")
                nc.vector.tensor_scalar_mul(tl[:], ident_h[:], cval)
                ident_cs[cval] = tl

            bxp_c = constp.tile([N, 1], F32, tag="bxp")
            nc.vector.tensor_scalar_mul(bxp_c[:], bx_c, STEP)

            # ------- setup phase: expm's + weight assembly -------
            # Weights for the x-form recurrence (all stored transposed, as
            # matmul lhsT):
            PTf = constp.tile([N, N], F32, tag="PTf")      # P^T fp32
            P2Tm = constp.tile([N, N], mdt, tag="P2Tm")    # (P M)^T
            PBTm = constp.tile([N, N], mdt, tag="PBTm")    # (P B')^T
            QTm = constp.tile([N, N], mdt, tag="QTm")      # Q^T
            A2Tm = constp.tile([N, N], mdt, tag="A2Tm")    # (0.01A M)^T
            ABTm = constp.tile([N, N], mdt, tag="ABTm")    # (0.01A B')^T
            BpTm = constp.tile([N, N], mdt, tag="BpTm")    # (0.01 B)^T
            A01Tm = constp.tile([N, N], mdt, tag="A01Tm")  # (0.01 A)^T
            CTf32 = constp.tile([N, N], F32, tag="CTf32")  # C^T fp32
            r_c = constp.tile([N, 1], F32, tag="rc")       # 0.01 C bx + by
            r2_c = constp.tile([N, 1], F32, tag="r2c")     # r + P bxp
            bx2_c = constp.tile([N, 1], F32, tag="bx2c")   # (I + 0.01A) bxp

            with (
                tc.tile_pool(name="zbuf", bufs=1) as zp,
                tc.tile_pool(name="work", bufs=2) as wp,
                tc.tile_pool(name="eres", bufs=1) as ep,
                tc.tile_pool(name="small", bufs=1) as sp,
                tc.tile_pool(name="pss", bufs=4, space="PSUM") as psp,
            ):
                zalls = zp.tile([N, 9 * N], F32, tag="zalls")
                nc.scalar.dma_start(
                    out=zalls[:, 0:5 * N], in_=zall[:, 0:5 * N]
                )
                nc.sync.dma_start(
                    out=zalls[:, ds(5 * N, 4 * N)], in_=zall[:, ds(5 * N, 4 * N)]
                )
                zt = {
                    name: zalls[:, ds(k * N, N)]
                    for k, name in enumerate(PARAM_NAMES)
                }

                def expm_batch(specs, side_work=()):
                    """Interleaved expm(skew(Z))^T for all matrices at once.

                    Maintains the (T, T^T) pair through Horner + squaring so
                    no PE transposes are needed: with negX = X^T = -X,
                      X @ T     = matmul(lhsT=negX, rhs=T)
                      T^T @ X^T = matmul(lhsT=T,    rhs=negX)
                    The five chains are advanced stage-by-stage so PE/DVE/ACT
                    work from different chains overlaps (a single sequential
                    chain is latency-bound on the MM->STT->MM dependency).
                    """
                    scal = 1.0 / (2.0 ** EXPM_SCAL)
                    negx = {}
                    t_cur = {}
                    tt_cur = {}
                    for z_tile, tag in specs:
                        us = wp.tile([N, N], FP16, tag="us_r", name=f"us_{tag}")
                        nc.vector.scalar_tensor_tensor(
                            us[:], z_tile, scal, masku,
                            op0=ALU.mult, op1=ALU.mult,
                        )
                        pst = psp.tile([N, N], FP16, tag="ps", bufs=8,
                                       name=f"pst_{tag}")
                        nc.tensor.transpose(pst[:], us[:], ident_h[:])
                        nx = wp.tile([N, N], FP16, tag=f"negx_{tag}", bufs=1,
                                     name=f"negx_{tag}")
                        nc.vector.scalar_tensor_tensor(
                            nx[:], pst[:], 1.0, us[:],
                            op0=ALU.mult, op1=ALU.subtract,
                        )
                        negx[tag] = nx
                        t_cur[tag] = ident_h
                        tt_cur[tag] = ident_h
                    # Integer-scaled Horner: w_k = c_k I + X w_{k-1}
                    # (w_0 = I, final w = c_last * T). The c_k I term comes
                    # from an identity matmul accumulated into the psum
                    # group, so each term needs only a plain psum->sbuf
                    # copy, alternated across DVE and ACT (DVE is the
                    # setup-phase bottleneck otherwise). The 1/c_last
                    # normalization folds into the first squaring's copy
                    # scale.
                    nsplit = 0
                    for k, j in enumerate(range(EXPM_TERMS, 0, -1)):
                        cval = horner_cs[k]
                        for _, tag in specs:
                            psa = psp.tile([N, N], F32, tag="ps", bufs=8)
                            nc.tensor.matmul(
                                psa[:], ident_h[:], ident_cs[cval][:],
                                start=True, stop=False,
                            )
                            nc.tensor.matmul(
                                psa[:], negx[tag][:], t_cur[tag][:],
                                start=False, stop=True,
                            )
                            t_new = wp.tile([N, N], FP16, tag=f"T_{tag}",
                                            bufs=2, name=f"T_{tag}")
                            if nsplit % 2 == 0:
                                nc.vector.tensor_copy(t_new[:], psa[:])
                            else:
                                nc.scalar.copy(t_new[:], psa[:])
                            nsplit += 1
                            t_cur[tag] = t_new
                        if side_work:
                            side_work.pop(0)()
                    for _, tag in specs:
                        pst = psp.tile([N, N], FP16, tag="ps", bufs=8,
                                       name=f"ptt_{tag}")
                        nc.tensor.transpose(pst[:], t_cur[tag][:], ident_h[:])
                        tt_new = wp.tile([N, N], FP16, tag=f"TT_{tag}",
                                         bufs=2, name=f"TT_{tag}")
                        nc.scalar.copy(tt_new[:], pst[:])
                        tt_cur[tag] = tt_new
                    wnorm = 1.0 / (horner_cs[-1] ** 2)
                    for sq in range(EXPM_SCAL):
                        for _, tag in specs:
                            psa = psp.tile([N, N], F32, tag="ps", bufs=8)
                            psb = psp.tile([N, N], F32, tag="ps", bufs=8)
                            nc.tensor.matmul(
                                psa[:], tt_cur[tag][:], t_cur[tag][:],
                                start=True, stop=True,
                            )
                            nc.tensor.matmul(
                                psb[:], t_cur[tag][:], tt_cur[tag][:],
                                start=True, stop=True,
                            )
                            t_new = wp.tile([N, N], FP16, tag=f"T_{tag}",
                                            bufs=2, name=f"T_{tag}")
                            tt_new = wp.tile([N, N], FP16, tag=f"TT_{tag}",
                                             bufs=2, name=f"TT_{tag}")
                            if sq == 0:
                                # both operands carried the c_last factor
                                nc.vector.tensor_scalar_mul(
                                    t_new[:], psa[:], wnorm
                                )
                                nc.scalar.activation(
                                    tt_new[:], psb[:], ACTF.Copy, scale=wnorm
                                )
                            else:
                                nc.vector.tensor_copy(t_new[:], psa[:])
                                nc.scalar.copy(tt_new[:], psb[:])
                            t_cur[tag], tt_cur[tag] = t_new, tt_new
                        if side_work:
                            side_work.pop(0)()
                    while side_work:
                        side_work.pop(0)()
                    return tt_cur

                # top-4 alpha pipeline (absdiag -> 4 max-extractions ->
                # sqrt): a serial DVE+GpSimd chain. Issued as side_work
                # hooks between expm Horner/squaring stages so it overlaps
                # the expm instead of stalling the A-matrix assembly after.
                dc_col = sp.tile([N, 1], F32, tag="dc")
                db_col = sp.tile([N, 1], F32, tag="db")
                ga_col = sp.tile([N, 1], F32, tag="ga")
                bwork = sp.tile([N, 1], F32, tag="bwork")
                cwork = sp.tile([N, 1], F32, tag="cwork")
                acc = sp.tile([N, 1], F32, tag="acc")
                bmax = sp.tile([N, 1], F32, tag="bmax")
                cmax = sp.tile([N, 1], F32, tag="cmax")
                prod = sp.tile([N, 1], F32, tag="prod")
                gmask = sp.tile([N, 1], F32, tag="gmask")
                tdrop = sp.tile([N, 1], F32, tag="tdrop")
                alpha = sp.tile([N, 1], F32, tag="alpha")
                sa05 = sp.tile([N, 1], F32, tag="sa05")
                sb01 = sp.tile([N, 1], F32, tag="sb01")

                def topk_absdiag():
                    for z_ap, col in (
                        (zt["ZC_S"], dc_col), (zt["ZB_S"], db_col),
                        (zt["ZA_G"], ga_col),
                    ):
                        tmp = wp.tile([N, N], F32, tag="us")
                        nc.vector.tensor_mul(tmp[:], z_ap, ident)
                        nc.vector.tensor_reduce(
                            col[:], tmp[:], AXIS.X, ALU.add,
                            apply_absolute_value=True,
                        )
                    nc.vector.tensor_copy(bwork[:], db_col[:])
                    nc.vector.tensor_copy(cwork[:], dc_col[:])
                    nc.vector.memset(acc[:], 0.0)

                def topk_iter(i):
                    def go():
                        nc.gpsimd.partition_all_reduce(
                            bmax[:], bwork[:], N, bass_isa.ReduceOp.max
                        )
                        nc.gpsimd.partition_all_reduce(
                            cmax[:], cwork[:], N, bass_isa.ReduceOp.max
                        )
                        nc.vector.tensor_mul(prod[:], bmax[:], cmax[:])
                        nc.vector.tensor_mul(prod[:], prod[:], prod[:])
                        nc.vector.tensor_add(acc[:], acc[:], prod[:])
                        if i < KTOP - 1:
                            # zero out the extracted max (values all > 0)
                            nc.vector.tensor_single_scalar(
                                gmask[:], bwork[:], bmax[:], ALU.is_ge
                            )
                            nc.vector.tensor_mul(tdrop[:], bwork[:], gmask[:])
                            nc.vector.tensor_sub(bwork[:], bwork[:], tdrop[:])
                            nc.vector.tensor_single_scalar(
                                gmask[:], cwork[:], cmax[:], ALU.is_ge
                            )
                            nc.vector.tensor_mul(tdrop[:], cwork[:], gmask[:])
                            nc.vector.tensor_sub(cwork[:], cwork[:], tdrop[:])
                    return go

                def topk_final():
                    # alpha = sqrt(sum_i (b_i c_i)^2) over the top-4 pairs
                    nc.scalar.activation(alpha[:], acc[:], ACTF.Sqrt)
                    # sa05 = -0.5*(alpha + gA) (per-partition row scale)
                    nc.vector.tensor_scalar(
                        sa05[:], ga_col[:], alpha[:], -0.5,
                        op0=ALU.add, op1=ALU.mult,
                    )
                    nc.vector.tensor_scalar_mul(sb01[:], db_col[:], STEP)

                eres = expm_batch(
                    [
                        (zt["ZC_U"], "UCT"), (zt["ZC_V"], "VCT"),
                        (zt["ZB_U"], "UBT"), (zt["ZB_V"], "VBT"),
                        (zt["ZA_U"], "UAT"),
                    ],
                    side_work=[topk_absdiag]
                    + [topk_iter(i) for i in range(KTOP)]
                    + [topk_final],
                )
                uct, vct = eres["UCT"], eres["VCT"]
                ubt, vbt = eres["UBT"], eres["VBT"]
                uat = eres["UAT"]

                # C^T = VC @ (SC @ UC^T)
                p1 = wp.tile([N, N], FP16, tag="us_r", name="p1")
                nc.vector.tensor_scalar_mul(p1[:], uct[:], dc_col[:])
                psa = psp.tile([N, N], F32, tag="ps", bufs=8)
                nc.tensor.matmul(psa[:], vct[:], p1[:], start=True, stop=True)
                nc.vector.tensor_copy(CTf32[:], psa[:])
                ct_h = ep.tile([N, N], FP16, tag="ct_h")
                nc.scalar.copy(ct_h[:], psa[:])

                # (0.01 B)^T = VB @ (0.01 SB @ UB^T)
                p2 = wp.tile([N, N], FP16, tag="us_r", name="p2")
                nc.vector.tensor_scalar_mul(p2[:], ubt[:], sb01[:])
                psb = psp.tile([N, N], F32, tag="ps", bufs=8)
                nc.tensor.matmul(psb[:], vbt[:], p2[:], start=True, stop=True)
                nc.vector.tensor_copy(BpTm[:], psb[:])
                # untransposed 0.01 B = UB @ (0.01 SB @ VB^T)
                p2b = wp.tile([N, N], FP16, tag="us_r", name="p2b")
                nc.vector.tensor_scalar_mul(p2b[:], vbt[:], sb01[:])
                psb2 = psp.tile([N, N], F32, tag="ps", bufs=8)
                nc.tensor.matmul(psb2[:], ubt[:], p2b[:], start=True, stop=True)
                bp_h = ep.tile([N, N], FP16, tag="Bpun")
                nc.vector.tensor_copy(bp_h[:], psb2[:])

                # M = UA @ (sa05 * UA^T) = 0.5*UA SA UA^T (symmetric)
                p3 = wp.tile([N, N], FP16, tag="us_r", name="p3")
                nc.vector.tensor_scalar_mul(p3[:], uat[:], sa05[:])
                psm = psp.tile([N, N], F32, tag="ps", bufs=8)
                nc.tensor.matmul(psm[:], uat[:], p3[:], start=True, stop=True)
                # YA = Uy - Uy^T; q2 = -0.005*YA
                uy = wp.tile([N, N], FP16, tag="us")
                nc.vector.tensor_mul(uy[:], zt["ZA_Y"], masku)
                pst2 = psp.tile([N, N], FP16, tag="ps", bufs=8)
                nc.tensor.transpose(pst2[:], uy[:], ident_h[:])
                nc.vector.tensor_scalar_mul(uy[:], uy[:], 0.5 * STEP)
                q2 = wp.tile([N, N], F32, tag="T")
                nc.vector.scalar_tensor_tensor(
                    q2[:], pst2[:], 0.5 * STEP, uy[:], op0=ALU.mult, op1=ALU.subtract
                )
                # (0.01 A)^T = 0.01*M + q2 ; untransposed 0.01 A = 0.01*M - q2
                a01tf = ep.tile([N, N], F32, tag="A01Tf")
                nc.vector.scalar_tensor_tensor(
                    a01tf[:], psm[:], STEP, q2[:], op0=ALU.mult, op1=ALU.add
                )
                nc.vector.tensor_copy(A01Tm[:], a01tf[:])
                a01t_h = ep.tile([N, N], FP16, tag="A01Th")
                nc.vector.tensor_copy(a01t_h[:], a01tf[:])
                a01h_un = ep.tile([N, N], FP16, tag="A01un")
                nc.vector.scalar_tensor_tensor(
                    a01h_un[:], psm[:], STEP, q2[:], op0=ALU.mult, op1=ALU.subtract
                )
                bxp_h = sp.tile([N, 1], FP16, tag="bxp_h")
                nc.vector.tensor_copy(bxp_h[:], bxp_c[:])

                # All weight-product matmuls run on fp16 operands (one FWL
                # weight load instead of fp32's two serialized 333ns LOW/
                # HIGH loads); the fp32 additive combines keep the dominant
                # terms exact, and everything here lands in bf16 anyway.
                # P^T = C^T + (0.01 A)^T C^T ;  Q^T = (0.01 B)^T C^T
                psw = psp.tile([N, N], F32, tag="ps", bufs=8)
                nc.tensor.matmul(psw[:], a01h_un[:], ct_h[:], start=True, stop=True)
                nc.vector.scalar_tensor_tensor(
                    PTf[:], psw[:], 1.0, CTf32[:], op0=ALU.mult, op1=ALU.add
                )
                pt_h = ep.tile([N, N], FP16, tag="pt_h")
                nc.vector.tensor_copy(pt_h[:], PTf[:])
                psq = psp.tile([N, N], F32, tag="ps", bufs=8)
                nc.tensor.matmul(psq[:], bp_h[:], ct_h[:], start=True, stop=True)
                nc.vector.tensor_copy(QTm[:], psq[:])

                # P2^T = P^T + (0.01A)^T P^T ; PB^T = (0.01B)^T P^T
                ps2 = psp.tile([N, N], F32, tag="ps", bufs=8)
                nc.tensor.matmul(ps2[:], a01h_un[:], pt_h[:], start=True, stop=True)
                nc.vector.scalar_tensor_tensor(
                    P2Tm[:], ps2[:], 1.0, PTf[:], op0=ALU.mult, op1=ALU.add
                )
                pspb = psp.tile([N, N], F32, tag="ps", bufs=8)
                nc.tensor.matmul(pspb[:], bp_h[:], pt_h[:], start=True, stop=True)
                nc.vector.tensor_copy(PBTm[:], pspb[:])

                # A2^T = (0.01A)^T + ((0.01A)^2)^T ; AB^T = (0.01B)^T (0.01A)^T
                psa2 = psp.tile([N, N], F32, tag="ps", bufs=8)
                nc.tensor.matmul(psa2[:], a01h_un[:], a01t_h[:], start=True, stop=True)
                nc.vector.scalar_tensor_tensor(
                    A2Tm[:], psa2[:], 1.0, a01tf[:], op0=ALU.mult, op1=ALU.add
                )
                psab = psp.tile([N, N], F32, tag="ps", bufs=8)
                nc.tensor.matmul(psab[:], bp_h[:], a01t_h[:], start=True, stop=True)
                nc.vector.tensor_copy(ABTm[:], psab[:])

                # r = 0.01 C bx + by ; r2 = r + P bxp ; bx2 = bxp + 0.01A bxp
                psr = psp.tile([N, 1], F32, tag="ps", bufs=8, name="psr")
                nc.tensor.matmul(psr[:], ct_h[:], bxp_h[:], start=True, stop=True)
                nc.vector.scalar_tensor_tensor(
                    r_c[:], psr[:], 1.0, by_c, op0=ALU.mult, op1=ALU.add
                )
                psr2 = psp.tile([N, 1], F32, tag="ps", bufs=8, name="psr2")
                nc.tensor.matmul(psr2[:], pt_h[:], bxp_h[:], start=True, stop=True)
                nc.vector.scalar_tensor_tensor(
                    r2_c[:], psr2[:], 1.0, r_c[:], op0=ALU.mult, op1=ALU.add
                )
                psbx = psp.tile([N, 1], F32, tag="ps", bufs=8, name="psbx")
                nc.tensor.matmul(psbx[:], a01t_h[:], bxp_h[:], start=True, stop=True)
                nc.vector.scalar_tensor_tensor(
                    bx2_c[:], psbx[:], 1.0, bxp_c[:], op0=ALU.mult, op1=ALU.add
                )

            # ------- recurrence (split-carry, x-form) -------
            with (
                tc.tile_pool(name="xrb", bufs=2) as xrbp,
                tc.tile_pool(name="xc", bufs=3) as xcp,
                tc.tile_pool(name="stage", bufs=2) as stagep,
                tc.tile_pool(name="th", bufs=3) as thp,
                tc.tile_pool(name="psy", bufs=3, space="PSUM") as psyp,
                tc.tile_pool(name="psx", bufs=3, space="PSUM") as psxp,
                tc.tile_pool(name="pstr", bufs=2, space="PSUM") as pstrp,
            ):
                # The rounded bf16 state copies live in a chunk buffer with
                # consecutive steps adjacent, so each pair-transpose covers
                # steps (s-1, s): transposes spread one-per-two-cycles over
                # the whole chunk, run in bf16 (FWL ident load, 1 cyc/row)
                # and so never crowd the chain-critical Q matmul. The fp32
                # carry never needs transposing and stays in a small pool.
                # th_0 = tanh(C x_0 + by), fp32 matmul (one-time)
                psy0 = psyp.tile([N, BSH], F32, tag="psy", name="psy0")
                nc.tensor.matmul(psy0[:], CTf32[:], x0_c, start=True, stop=True)
                th_cur = thp.tile([N, BSH], mdt, tag="th", name="th_init")
                nc.scalar.activation(
                    th_cur[:], psy0[:], ACTF.Tanh, bias=by_c, scale=1.0
                )
                xc_prev = xcp.tile([N, BSH], F32, tag="xc", name="xc0")
                nc.vector.tensor_copy(xc_prev[:], x0_c)
                xrb0 = xrbp.tile([N, tc_chunk * BSH], mdt, tag="xrb")
                nc.vector.tensor_copy(xrb0[:, 0:BSH], x0_c)
                xr_old = xrb0[:, 0:BSH]  # xr_{t-1} at iteration t=1
                # psy_1 = P x_0 + Q th_0 (P-form, one-time; P @ x_0 in fp32)
                psy_t = psyp.tile([N, BSH], F32, tag="psy", name="psy1")
                nc.tensor.matmul(psy_t[:], PTf[:], x0_c, start=True, stop=False)
                nc.tensor.matmul(psy_t[:], QTm[:], th_cur[:], start=False, stop=True)
                # psx_1 = 0.01A x_0 + 0.01B th_0 (P-form, one-time)
                psx_t = psxp.tile([N, BSH], F32, tag="psx", name="psx1")
                nc.tensor.matmul(psx_t[:], A01Tm[:], xr_old, start=True, stop=False)
                nc.tensor.matmul(psx_t[:], BpTm[:], th_cur[:], start=False, stop=True)

                th_old = th_cur  # th_0 (th_{t-1} at iteration t=1)

                xrb_prev = None
                for c in range(nchunks):
                    xrb = xrb0 if c == 0 else xrbp.tile(
                        [N, tc_chunk * BSH], mdt, tag="xrb"
                    )
                    st = stagep.tile([128, half * N], F32, tag="st")
                    for s in range(tc_chunk):
                        t = c * tc_chunk + s
                        if t > 0:
                            xslot = xrb[:, ds(s * BSH, BSH)]
                            # chain-critical: th_t = tanh(psy_t + r)
                            th_new = thp.tile([N, BSH], mdt, tag="th")
                            nc.scalar.activation(
                                th_new[:], psy_t[:], ACTF.Tanh,
                                bias=(r_c[:] if t == 1 else r2_c[:]), scale=1.0,
                            )
                            # exact fp32 state carry
                            xc_new = xcp.tile([N, BSH], F32, tag="xc")
                            nc.vector.scalar_tensor_tensor(
                                xc_new[:], psx_t[:],
                                bxp_c[:] if t == 1 else bx2_c[:],
                                xc_prev[:], op0=ALU.add, op1=ALU.add,
                            )
                            xc_prev = xc_new
                            # rounded copy for matmuls + output transposes
                            # (cheap SBUF->SBUF DVE op, off the chain;
                            # NOT on Pool -- a Pool cast costs ~450ns and
                            # paces the whole loop to ~930ns/step)
                            nc.vector.tensor_copy(xslot, xc_new[:])
                            xr_new = xslot
                            if t < tmax - 1:
                                # psy_{t+1} / psx_{t+1}: the four t-1
                                # partials execute during tanh_t; only
                                # Q@th_t / B'@th_t wait on the tanh.
                                psy_n = psyp.tile([N, BSH], F32, tag="psy")
                                psx_n = psxp.tile([N, BSH], F32, tag="psx")
                                nc.tensor.matmul(
                                    psy_n[:], P2Tm[:], xr_old,
                                    start=True, stop=False,
                                )
                                nc.tensor.matmul(
                                    psx_n[:], A2Tm[:], xr_old,
                                    start=True, stop=False,
                                )
                                nc.tensor.matmul(
                                    psy_n[:], PBTm[:], th_old[:],
                                    start=False, stop=False,
                                )
                                nc.tensor.matmul(
                                    psx_n[:], ABTm[:], th_old[:],
                                    start=False, stop=False,
                                )
                                nc.tensor.matmul(
                                    psy_n[:], QTm[:], th_new[:],
                                    start=False, stop=True,
                                )
                                nc.tensor.matmul(
                                    psx_n[:], BpTm[:], th_new[:],
                                    start=False, stop=True,
                                )
                                psy_t, psx_t = psy_n, psx_n
                            xr_old = xr_new
                            th_old = th_new
                        if s % 2 == 1:
                            j = s // 2
                            # transpose steps (s-1, s): one contiguous
                            # 128-col block. Two pair-transposes share one
                            # psum tile; a single DVE copy drains both
                            # (keeps the drain off ACT so it can't
                            # queue-block the chain-critical tanh).
                            if j % 2 == 0:
                                pstr = pstrp.tile([128, 2 * N], mdt, tag="pstr")
                            nc.tensor.transpose(
                                pstr[:, ds((j % 2) * N, N)],
                                xrb[:, ds((s - 1) * BSH, 2 * BSH)],
                                ident_b[:],
                            )
                            if j % 2 == 1 or s == tc_chunk - 1:
                                lo = (j - (j % 2)) * N
                                width = (j % 2 + 1) * N
                                nc.vector.tensor_copy(
                                    st[:, ds(lo, width)], pstr[:, 0:width]
                                )
                    # st col-block j holds steps (2j, 2j+1): partitions
                    # 0-63 = even step's batch, 64-127 = odd step's, so a
                    # partition-half DMA writes stride-2 time indices.
                    qn = max(half // 4, 1)
                    ndma = 0
                    for h in range(2):
                        for q0 in range(0, half, qn):
                            t0 = c * tc_chunk + h + 2 * q0
                            eng = nc.sync
                            if c == nchunks - 1 and ndma % 2 == 1:
                                eng = nc.scalar
                            eng.dma_start(
                                out=out[:, t0:t0 + 2 * qn - 1:2, :],
                                in_=st[h * 64:(h + 1) * 64, ds(q0 * N, qn * N)],
                            )
                            ndma += 1
                    xrb_prev = xrb

    nc.compile()
    return nc


_CACHED = {}


def _get_program(tmax=TMAX, tc_chunk=32, mdt=BF16):
    key = (tmax, tc_chunk, str(mdt))
    if key not in _CACHED:
        _CACHED[key] = build_program(tmax, tc_chunk, mdt)
    return _CACHED[key]


def make_in_maps(inputs, tmax=TMAX):
    X0 = np.ascontiguousarray(np.asarray(inputs["X0"], dtype=np.float32))
    zall = np.concatenate(
        [np.asarray(inputs[name], dtype=np.float32) for name in PARAM_NAMES],
        axis=1,
    )
    base = {"zall": np.ascontiguousarray(zall)}
    in_maps = []
    for c in range(NCORES):
        m = dict(base)
        misc = np.concatenate(
            [
                np.eye(N, dtype=np.float32),
                np.triu(np.ones((N, N), dtype=np.float32), 1),
                X0[c * BSH:(c + 1) * BSH].T,
                np.asarray(inputs["bx"], dtype=np.float32).reshape(N, 1),
                np.asarray(inputs["by"], dtype=np.float32).reshape(N, 1),
            ],
            axis=1,
        )
        m["misc"] = np.ascontiguousarray(misc)
        in_maps.append(m)
    return in_maps


def run_spmd(inputs, tmax=TMAX, tc_chunk=32, trace=False, tmpdir=None, mdt=BF16):
    nc = _get_program(tmax, tc_chunk, mdt)
    in_maps = make_in_maps(inputs, tmax)
    res = run_bass_kernel_spmd(
        nc, in_maps, list(range(NCORES)), trace=trace, tmpdir=tmpdir
    )
    outs = [res.results[c]["out"] for c in range(NCORES)]
    full = np.concatenate(outs, axis=0)
    return full, res


def kernel(**inputs):
    full, _ = run_spmd(inputs)
    return full
